# revision 60
# baseline (speedup 1.0000x reference)
"""Trainium2 Bass kernel for banded (sliding-window) single-head attention.

Problem (hardcoded):
    x     [256, 256, 768] f32   (batch, tokens, dim)
    w_qkv [768, 192]      f32
    w_out [64, 768]       f32
    b_out [768]           f32
    y = (softmax(band_mask(q k^T / 8)) v) @ w_out + b_out,  band |i-j| < 32

Strategy: pure data parallel over batch (32 batches/core on 8 cores).

Per-core kernel design (fp16 operands, fp32 PSUM accumulation):
  - x arrives as a single fp16 plane, PRE-TRANSPOSED on the host into a
    PARTITION-MAJOR layout x_blk[p, (pt, half, c3, t)] = cast(x)[pt*PT + t,
    (half*3 + c3)*128 + p], so each input DMA is a plain linear load with
    3 KB contiguous per partition row.  All constants (w_qkv chunk-blocked,
    [w_out; b_out] stack, compact band masks, transpose identity) are
    host-prebuilt and land in a few linear DMAs -- no on-device
    iota/affine_select/cast preamble.
  - SOFTWARE PIPELINE over ptiles (512 tokens = 2 batches), tails running
    TWO ptiles behind the fronts.  Per iteration the emission order is
    proj(pt) -> finals-2nd-half(pt-3) -> PV(pt-2) -> attn(pt) ->
    finals-1st-half(pt-2), tuned so every PSUM-bank reuse and every
    cross-engine operand has at least a section of slack.
  - COMPACT banded layout: scores for both j-chunks of a batch live in
    one psum bank [128, 2, 160] f32 whose tail 128 bytes (bitcast fp16)
    receive the PE transposes of v for that batch -- softmax is a single
    wide exp ACT per batch and the band mask is a gpsimd multiply
    (scalar/vector, the only engines with PSUM ports, never touch it).
    PV is 3 column-range matmuls per batch (160 | 64 accumulate | 96).
  - UNNORMALIZED output: o_aug [65, t] = v_aug^T @ pexp (row 64 = softmax
    sums via the ones column of v_aug) is evicted once per ptile as
    osc [65, 512]; the final projection runs TRANSPOSED, yT[dc 128, t] =
    waug_dc^T @ osc with waug = [w_out; b_out] chunk as a CONSTANT lhsT
    (LDWEIGHTS never waits on data) -- the b_out row times the sums row
    adds bias*sums, exact after normalization.  y ships as fp16 yT plus
    the raw sums row (one tiny DMA per ptile); the HOST divides by the
    sums and casts -- no reciprocal, no per-partition scale fusion, no
    normalization serialization on device.
  - PSUM is exactly 8 banks: qk/v projection ring 2, scores(+vT) ring 2,
    and a 4-bank ring holding the PV output and the 6 final-projection
    banks (7 allocations/iteration -> every reuse has >=1.5 us of
    eviction slack).
"""

import numpy as np

import concourse.mybir as mybir
import concourse.tile as tile
from concourse import bacc
from concourse import bass_utils

F32 = mybir.dt.float32

DT = mybir.dt.float16
NP_DT = np.float16

B, N, D, DH = 256, 256, 768, 64
SA = 32                       # band half-width: |i-j| < SA
NCORES = 8
BLOC = B // NCORES            # batches per core
TOK_FULL = BLOC * N           # tokens per core (8192)
PT = 512                      # tokens per pipeline tile (2 batches)
NC_CHUNKS = D // 128          # 6 contraction chunks
BW = 160                      # banded column range width per j-chunk


def build_body(tc, x_blk, wq_blk, waug_in, maskc_in, ident_in, y, sums_out,
               tok, ctx):
    nc = tc.nc
    npt = tok // PT
    nbatch_pt = PT // N       # batches per ptile (2)

    const = ctx.enter_context(tc.tile_pool(name="const", bufs=1))
    xplane_pool = ctx.enter_context(tc.tile_pool(name="xplane", bufs=5))
    qkv_pool = ctx.enter_context(tc.tile_pool(name="qkv", bufs=3))
    vaug_pool = ctx.enter_context(tc.tile_pool(name="vaug", bufs=6))
    exp_pool = ctx.enter_context(tc.tile_pool(name="exp", bufs=8))
    osc_pool = ctx.enter_context(tc.tile_pool(name="osc", bufs=4))
    y_pool = ctx.enter_context(tc.tile_pool(name="ysb", bufs=4))

    # PSUM: 8 banks total.  proj ring 2 holds qk/v (casts run first each
    # iteration, giving a full period of slack); sc ring 2 holds scores
    # AND the transposed-v spare region (bitcast fp16); f ring 4 holds
    # the PV output o plus the 6 final-projection banks -- 7 allocations
    # per iteration through 4 banks gives every reuse >=1.5us of
    # eviction slack.
    ps_proj = ctx.enter_context(tc.tile_pool(name="psproj", bufs=2, space="PSUM"))
    ps_sc = ctx.enter_context(tc.tile_pool(name="pssc", bufs=2, space="PSUM"))
    ps_f = ctx.enter_context(tc.tile_pool(name="psf", bufs=4, space="PSUM"))

    # ---- constants: linear DMAs, all host-prebuilt; chunk 0 of w first
    # so the first QKV matmul's LDWEIGHTS can start as early as possible
    wq_sb = const.tile([128, NC_CHUNKS, 192], DT)
    nc.scalar.dma_start(out=wq_sb[:, 0, :], in_=wq_blk[:, 0:192])
    nc.scalar.dma_start(
        out=wq_sb[:, 1:NC_CHUNKS, :],
        in_=wq_blk[:, 192:NC_CHUNKS * 192].rearrange(
            "p (c e) -> p c e", c=NC_CHUNKS - 1))
    waug = const.tile([65, D], DT)
    nc.scalar.dma_start(out=waug[:], in_=waug_in[:, :])
    maskc = const.tile([128, 2, BW], DT)
    nc.scalar.dma_start(out=maskc[:], in_=maskc_in[:, :, :])
    ident = const.tile([64, 64], DT)
    nc.scalar.dma_start(out=ident[:], in_=ident_in[:, :])

    def emit_proj(pt):
        """x load, QKV projection and PSUM->SBUF casts.

        Emitted FIRST in each iteration so the casts sit ahead of the
        previous tail's y-copies in the scalar/vector queues -- the
        casts feed the PE's next scores/transposes, while y-copies only
        feed the (slack) output DMA.  This breaks the serial cycle
        finals -> y-copies -> casts -> scores.
        """
        # linear 16-bit load: xp[p, c, t] = x_blk[(pt*6 + c)*128 + p, t]
        xp = xplane_pool.tile([128, 2, NC_CHUNKS // 2, PT], DT, tag="xp")
        hlen = NC_CHUNKS // 2 * PT
        if pt == 0:
            # chunk-granular first load so QKV chunk 0 starts sooner
            for h in range(2):
                for c3 in range(NC_CHUNKS // 2):
                    o0 = (pt * 2 + h) * hlen + c3 * PT
                    nc.sync.dma_start(
                        out=xp[:, h, c3, :],
                        in_=x_blk[:, o0:o0 + PT],
                    )
        else:
            for h in range(2):
                o0 = (pt * 2 + h) * hlen
                nc.sync.dma_start(
                    out=xp[:, h, :, :],
                    in_=x_blk[:, o0:o0 + hlen].rearrange(
                        "p (c t) -> p c t", c=NC_CHUNKS // 2),
                )

        # qkT: [128(e = q|k), PT] = sum_c W_qk[c].T @ x[c]
        qk_ps = ps_proj.tile([128, PT], F32, tag="proj")
        v_ps = ps_proj.tile([128, PT], F32, tag="proj")
        if pt == 0:
            # warmup: interleave qk/v per chunk so each freshly-landed x
            # chunk feeds two matmuls while the next chunk's DMA streams
            for c in range(NC_CHUNKS):
                nc.tensor.matmul(
                    qk_ps[:], lhsT=wq_sb[:, c, 0:128],
                    rhs=xp[:, c // 3, c % 3, :],
                    start=(c == 0), stop=(c == NC_CHUNKS - 1),
                )
                nc.tensor.matmul(
                    v_ps[0:64, :], lhsT=wq_sb[:, c, 128:192],
                    rhs=xp[:, c // 3, c % 3, :],
                    start=(c == 0), stop=(c == NC_CHUNKS - 1),
                )
        else:
            for c in range(NC_CHUNKS):
                nc.tensor.matmul(
                    qk_ps[:], lhsT=wq_sb[:, c, 0:128],
                    rhs=xp[:, c // 3, c % 3, :],
                    start=(c == 0), stop=(c == NC_CHUNKS - 1),
                )
            for c in range(NC_CHUNKS):
                nc.tensor.matmul(
                    v_ps[0:64, :], lhsT=wq_sb[:, c, 128:192],
                    rhs=xp[:, c // 3, c % 3, :],
                    start=(c == 0), stop=(c == NC_CHUNKS - 1),
                )
        qT = qkv_pool.tile([64, PT], DT, tag="qT")
        kT = qkv_pool.tile([64, PT], DT, tag="kT")
        nc.scalar.copy(qT[:], qk_ps[0:64, :])
        nc.vector.tensor_copy(kT[:], qk_ps[64:128, :])
        vT = qkv_pool.tile([64, PT], DT, tag="vT")
        nc.vector.tensor_copy(vT[:], v_ps[0:64, :])

        return {"pt": pt, "qT": qT, "kT": kT, "vT": vT}

    def emit_attn(fr):
        """vT transposes, banded scores, exp, mask for a projected ptile."""
        qT, kT, vT = fr["qT"], fr["kT"], fr["vT"]

        # per-batch PSUM tile: scores [:, jc, 0:160] f32 + transposed-v
        # spare [:, jc, 160:192] (bitcast fp16) in the SAME bank
        vaug = vaug_pool.tile([128, nbatch_pt, 2, 65], DT, tag="vaug")
        nc.gpsimd.memset(vaug[:, :, :, 64:65], 1.0)
        sc_list = [ps_sc.tile([128, 2, BW + 32], F32, tag="sc", name=f"sc{bb}")
                   for bb in range(nbatch_pt)]
        # all transposes first, then all scores: uniform PE tile-size runs
        for bb in range(nbatch_pt):
            for jc in range(2):
                nc.tensor.transpose(
                    sc_list[bb][:, jc, BW:BW + 32].bitcast(DT),
                    vT[:, bb * N + jc * 128: bb * N + (jc + 1) * 128],
                    ident[:],
                )
        for bb in range(nbatch_pt):
            t0 = bb * N
            # banded scores, compact:
            #   jc0: keys [0,128)   x queries [0,160)
            #   jc1: keys [128,256) x queries [96,256)
            for jc, ilo in ((0, 0), (1, N - BW)):
                nc.tensor.matmul(
                    sc_list[bb][:, jc, 0:BW],
                    lhsT=kT[:, t0 + jc * 128: t0 + (jc + 1) * 128],
                    rhs=qT[:, t0 + ilo: t0 + ilo + BW], start=True, stop=True,
                )
            nc.vector.tensor_copy(vaug[:, bb, :, 0:64],
                                  sc_list[bb][:, :, BW:BW + 32].bitcast(DT))

        # one wide exp per batch, then one gpsimd band-mask multiply
        pexps = []
        for bb in range(nbatch_pt):
            pexp = exp_pool.tile([128, 2, BW], DT, tag="pexp")
            nc.scalar.activation(
                pexp[:], sc_list[bb][:, :, 0:BW],
                mybir.ActivationFunctionType.Exp, scale=float(DH) ** -0.5,
            )
            pexps.append(pexp)
        # per-jc mask halves: PV's first matmuls only need jc0 masked
        for bb in range(nbatch_pt):
            nc.gpsimd.tensor_mul(pexps[bb][:, 0, :], pexps[bb][:, 0, :],
                                 maskc[:, 0, :])
        for bb in range(nbatch_pt):
            nc.gpsimd.tensor_mul(pexps[bb][:, 1, :], pexps[bb][:, 1, :],
                                 maskc[:, 1, :])

        fr["vaug"] = vaug
        fr["pexps"] = pexps
        return fr

    def emit_tail_pv(st, final=False):
        """PV matmuls, osc eviction, sums store for a previous ptile.

        o_ps lives in the f-ring (NOT the qk/v ring): PV must never wait
        on the current ptile's qk eviction casts.
        """
        pt, vaug, pexps = st["pt"], st["vaug"], st["pexps"]
        o_ps = ps_f.tile([128, PT], F32, tag="f", name="o_ps")
        for bb in range(nbatch_pt):
            t0 = bb * N
            pexp = pexps[bb]
            # compact banded PV, 3 matmuls: jc0 [0:160), jc1 accumulates
            # the overlap [96:160) (skip_group_check: the group already
            # stopped -- hardware accumulate is just write-with-add),
            # jc1 fresh [160:256)
            nc.tensor.matmul(
                o_ps[0:65, t0 + 0:t0 + 160],
                lhsT=vaug[:, bb, 0, :], rhs=pexp[:, 0, 0:BW],
                start=True, stop=True,
            )
            nc.tensor.matmul(
                o_ps[0:65, t0 + 96:t0 + 160],
                lhsT=vaug[:, bb, 1, :], rhs=pexp[:, 1, 0:64],
                start=False, stop=True, skip_group_check=True,
            )
            nc.tensor.matmul(
                o_ps[0:65, t0 + 160:t0 + 256],
                lhsT=vaug[:, bb, 1, :], rhs=pexp[:, 1, 64:BW],
                start=True, stop=True,
            )

        # single wide osc eviction, alternating engine; split across both
        # engines at the pipeline drain (nothing to hide behind there)
        osc = osc_pool.tile([65, PT], DT, tag="osc")
        if final:
            nc.scalar.copy(osc[:, 0:PT // 2], o_ps[0:65, 0:PT // 2])
            nc.vector.tensor_copy(osc[:, PT // 2:PT], o_ps[0:65, PT // 2:PT])
        elif pt % 2 == 0:
            nc.scalar.copy(osc[:], o_ps[0:65, :])
        else:
            nc.vector.tensor_copy(osc[:], o_ps[0:65, :])
        # softmax sums (osc row 64) ship raw; the host normalizes
        nc.sync.dma_start(out=sums_out[:, pt * PT:(pt + 1) * PT],
                          in_=osc[64:65, :])
        st["osc"] = osc

    def emit_tail_f(st, half, final=False):
        """Final projection, transposed: yT[dc, t] = waug_c^T @ osc.

        waug chunks are CONSTANT lhsT weights (LDWEIGHTS never waits on
        data), all 6 matmuls are uniform full-bank 512-column ops, and
        the bias-times-sums trick still applies (host divides by sums).
        Emitted in two halves one iteration apart so the f-ring bank
        evictions always have a full qk/v section to drain behind.
        """
        pt, osc = st["pt"], st["osc"]
        if half == 0:
            y_sb = y_pool.tile([128, NC_CHUNKS, PT], DT, tag="ysb")
            st["y_sb"] = y_sb
            dcs = range(0, NC_CHUNKS // 2)
        else:
            y_sb = st["y_sb"]
            dcs = range(NC_CHUNKS // 2, NC_CHUNKS)
        for dc in dcs:
            f_ps = ps_f.tile([128, PT], F32, tag="f")
            nc.tensor.matmul(
                f_ps[:], lhsT=waug[:, dc * 128:(dc + 1) * 128],
                rhs=osc[:], start=True, stop=True,
            )
            if final:
                # drain: split each eviction across both engines and DMA
                # each chunk as soon as it lands, so the last transfer is
                # small and the receipt latency shrinks
                nc.scalar.copy(y_sb[:, dc, 0:PT // 2], f_ps[:, 0:PT // 2])
                nc.vector.tensor_copy(y_sb[:, dc, PT // 2:PT],
                                      f_ps[:, PT // 2:PT])
                eng = nc.scalar if dc == NC_CHUNKS - 1 else nc.sync
                eng.dma_start(
                    out=y[:, (pt * NC_CHUNKS + dc) * PT:
                          (pt * NC_CHUNKS + dc + 1) * PT],
                    in_=y_sb[:, dc, :],
                )
            elif dc % 2 == 0:
                nc.scalar.copy(y_sb[:, dc, :], f_ps[:])
            else:
                nc.vector.tensor_copy(y_sb[:, dc, :], f_ps[:])
        if half == 1 and not final:
            eng = nc.sync
            eng.dma_start(
                out=y[:, pt * NC_CHUNKS * PT:
                      (pt + 1) * NC_CHUNKS * PT].rearrange(
                    "p (c t) -> p c t", c=NC_CHUNKS),
                in_=y_sb[:],
            )

    # ---- software-pipelined main loop ----
    # Per iteration: proj(pt) [qk/v matmuls + casts], finals 2nd half
    # (pt-3), tail_pv(pt-2) [PV + osc + sums], attn(pt) [transposes,
    # scores, exp, mask], finals 1st half (pt-2).
    sts = [None] * npt
    for pt in range(npt):
        sts[pt] = emit_proj(pt)
        if pt >= 3:
            emit_tail_f(sts[pt - 3], half=1)
        if pt >= 2:
            emit_tail_pv(sts[pt - 2])
        emit_attn(sts[pt])
        if pt >= 2:
            emit_tail_f(sts[pt - 2], half=0)
    # drain: PV groups as early as possible (osc evictions overlap the
    # other ptile's PE work), then the final-projection groups
    emit_tail_pv(sts[npt - 2])
    emit_tail_f(sts[npt - 3], half=1)
    emit_tail_f(sts[npt - 2], half=0)
    emit_tail_pv(sts[npt - 1], final=True)
    emit_tail_f(sts[npt - 2], half=1)
    emit_tail_f(sts[npt - 1], half=0, final=True)
    emit_tail_f(sts[npt - 1], half=1, final=True)


def build_nc(tok=TOK_FULL):
    nc = bacc.Bacc("TRN2", target_bir_lowering=False, debug=False)
    # x 16-bit, host-pre-transposed, chunk-blocked per ptile:
    # x_blk[(pt*6 + c)*128 + p, t] = cast(x)[pt*PT + t, c*128 + p]
    x_blk = nc.dram_tensor(
        "x_blk", [128, tok // PT * 2 * NC_CHUNKS // 2 * PT], DT,
        kind="ExternalInput").ap()
    wq_blk = nc.dram_tensor("wq_blk", [128, NC_CHUNKS * 192], DT,
                            kind="ExternalInput").ap()
    waug_in = nc.dram_tensor("waug", [65, D], DT, kind="ExternalInput").ap()
    maskc_in = nc.dram_tensor("maskc", [128, 2, BW], DT,
                              kind="ExternalInput").ap()
    ident_in = nc.dram_tensor("ident", [64, 64], DT, kind="ExternalInput").ap()
    # y is stored TRANSPOSED: y_blk[p, (pt, c, t)] = yraw[pt*PT+t, c*128+p]
    y = nc.dram_tensor("y", [128, tok // PT * NC_CHUNKS * PT], DT,
                       kind="ExternalOutput").ap()
    sums_out = nc.dram_tensor("sums", [1, tok], DT, kind="ExternalOutput").ap()

    from contextlib import ExitStack
    with tile.TileContext(nc) as tc:
        with ExitStack() as ctx:
            build_body(tc, x_blk, wq_blk, waug_in, maskc_in, ident_in, y,
                       sums_out, tok, ctx)
    nc.compile()
    return nc


def make_in_maps(x, w_qkv, w_out, b_out):
    w16 = np.asarray(w_qkv, dtype=NP_DT)
    wq_blk = np.ascontiguousarray(
        w16.reshape(NC_CHUNKS, 128, 192).transpose(1, 0, 2)
    ).reshape(128, NC_CHUNKS * 192)

    waug = np.zeros((65, D), dtype=NP_DT)
    waug[0:64, :] = np.asarray(w_out, dtype=NP_DT)
    waug[64, :] = np.asarray(b_out, dtype=NP_DT)

    # band masks on the compact column ranges:
    #   jc0: key j = k,     query i = c      -> keep iff |c - k| < SA
    #   jc1: key j = 128+k, query i = 96+c   -> keep iff |c - k - 32| < SA
    k = np.arange(128)[:, None]
    c = np.arange(BW)[None, :]
    maskc = np.zeros((128, 2, BW), dtype=NP_DT)
    maskc[:, 0, :] = (np.abs(c - k) < SA).astype(NP_DT)
    maskc[:, 1, :] = (np.abs(c - k - 32) < SA).astype(NP_DT)

    ident = np.eye(64, dtype=NP_DT)

    npt = TOK_FULL // PT
    in_maps = []
    for cc in range(NCORES):
        xc = np.asarray(x)[cc * BLOC:(cc + 1) * BLOC].reshape(TOK_FULL, D)
        xc16 = xc.astype(NP_DT)
        blk = np.ascontiguousarray(
            xc16.reshape(npt, PT, 2, NC_CHUNKS // 2, 128)
            .transpose(4, 0, 2, 3, 1)
        ).reshape(128, -1)
        in_maps.append({
            "x_blk": blk,
            "wq_blk": wq_blk, "waug": waug, "maskc": maskc, "ident": ident,
        })
    return in_maps


_NC_CACHE = {}


def run(x, w_qkv, w_out, b_out, trace=False, **trace_kwargs):
    if "nc" not in _NC_CACHE:
        _NC_CACHE["nc"] = build_nc()
    nc = _NC_CACHE["nc"]
    in_maps = make_in_maps(x, w_qkv, w_out, b_out)
    res = bass_utils.run_bass_kernel_spmd(
        nc, in_maps, core_ids=list(range(NCORES)), trace=trace, **trace_kwargs
    )
    npt = TOK_FULL // PT
    outs = []
    for c in range(NCORES):
        # y arrives transposed: [p, pt, dc, t] = yraw[pt*PT+t, dc*128+p]
        raw = (np.asarray(res.results[c]["y"], dtype=np.float32)
               .reshape(128, npt, NC_CHUNKS, PT).transpose(1, 3, 2, 0)
               .reshape(TOK_FULL, D))
        sums = np.asarray(res.results[c]["sums"], dtype=np.float32).reshape(
            TOK_FULL, 1)
        outs.append((raw / sums).reshape(BLOC, N, D))
    y = np.concatenate(outs, axis=0)
    return y, res


def kernel(x, w_qkv, w_out, b_out):
    y, _ = run(np.asarray(x), np.asarray(w_qkv), np.asarray(w_out),
               np.asarray(b_out))
    return y


# revision 61
# speedup vs baseline: 1.0096x; 1.0096x over previous
"""Trainium2 Bass kernel for banded (sliding-window) single-head attention.

Problem (hardcoded):
    x     [256, 256, 768] f32   (batch, tokens, dim)
    w_qkv [768, 192]      f32
    w_out [64, 768]       f32
    b_out [768]           f32
    y = (softmax(band_mask(q k^T / 8)) v) @ w_out + b_out,  band |i-j| < 32

Strategy: pure data parallel over batch (32 batches/core on 8 cores).

Per-core kernel design (fp16 operands, fp32 PSUM accumulation):
  - x arrives as a single fp16 plane, PRE-TRANSPOSED on the host into a
    PARTITION-MAJOR layout x_blk[p, (pt, half, c3, t)] = cast(x)[pt*PT + t,
    (half*3 + c3)*128 + p], so each input DMA is a plain linear load with
    3 KB contiguous per partition row.  All constants (w_qkv chunk-blocked,
    [w_out; b_out] stack, compact band masks, transpose identity) are
    host-prebuilt and land in a few linear DMAs -- no on-device
    iota/affine_select/cast preamble.
  - SOFTWARE PIPELINE over ptiles (512 tokens = 2 batches), tails running
    TWO ptiles behind the fronts.  Per iteration the emission order is
    proj(pt) -> finals-2nd-half(pt-3) -> PV(pt-2) -> attn(pt) ->
    finals-1st-half(pt-2), tuned so every PSUM-bank reuse and every
    cross-engine operand has at least a section of slack.
  - COMPACT banded layout: scores for both j-chunks of a batch live in
    one psum bank [128, 2, 160] f32 whose tail 128 bytes (bitcast fp16)
    receive the PE transposes of v for that batch -- softmax is a single
    wide exp ACT per batch and the band mask is a gpsimd multiply
    (scalar/vector, the only engines with PSUM ports, never touch it).
    PV is 3 column-range matmuls per batch (160 | 64 accumulate | 96).
  - UNNORMALIZED output: o_aug [65, t] = v_aug^T @ pexp (row 64 = softmax
    sums via the ones column of v_aug) is evicted once per ptile as
    osc [65, 512]; the final projection runs TRANSPOSED, yT[dc 128, t] =
    waug_dc^T @ osc with waug = [w_out; b_out] chunk as a CONSTANT lhsT
    (LDWEIGHTS never waits on data) -- the b_out row times the sums row
    adds bias*sums, exact after normalization.  y ships as fp16 yT plus
    the raw sums row (one tiny DMA per ptile); the HOST divides by the
    sums and casts -- no reciprocal, no per-partition scale fusion, no
    normalization serialization on device.
  - PSUM is exactly 8 banks: qk/v projection ring 2, scores(+vT) ring 2,
    and a 4-bank ring holding the PV output and the 6 final-projection
    banks (7 allocations/iteration -> every reuse has >=1.5 us of
    eviction slack).
"""

import numpy as np

import concourse.mybir as mybir
import concourse.tile as tile
from concourse import bacc
from concourse import bass_utils

F32 = mybir.dt.float32

DT = mybir.dt.float16
NP_DT = np.float16

B, N, D, DH = 256, 256, 768, 64
SA = 32                       # band half-width: |i-j| < SA
NCORES = 8
BLOC = B // NCORES            # batches per core
TOK_FULL = BLOC * N           # tokens per core (8192)
PT = 512                      # tokens per pipeline tile (2 batches)
NC_CHUNKS = D // 128          # 6 contraction chunks
BW = 160                      # banded column range width per j-chunk


def build_body(tc, x_blk, wq_blk, waug_in, maskc_in, ident_in, y, sums_out,
               tok, ctx):
    nc = tc.nc
    npt = tok // PT
    nbatch_pt = PT // N       # batches per ptile (2)

    const = ctx.enter_context(tc.tile_pool(name="const", bufs=1))
    xplane_pool = ctx.enter_context(tc.tile_pool(name="xplane", bufs=5))
    qkv_pool = ctx.enter_context(tc.tile_pool(name="qkv", bufs=3))
    vaug_pool = ctx.enter_context(tc.tile_pool(name="vaug", bufs=6))
    exp_pool = ctx.enter_context(tc.tile_pool(name="exp", bufs=8))
    osc_pool = ctx.enter_context(tc.tile_pool(name="osc", bufs=4))
    y_pool = ctx.enter_context(tc.tile_pool(name="ysb", bufs=4))

    # PSUM: 8 banks total.  proj ring 2 holds qk/v (casts run first each
    # iteration, giving a full period of slack); sc ring 2 holds scores
    # AND the transposed-v spare region (bitcast fp16); f ring 4 holds
    # the PV output o plus the 6 final-projection banks -- 7 allocations
    # per iteration through 4 banks gives every reuse >=1.5us of
    # eviction slack.
    ps_proj = ctx.enter_context(tc.tile_pool(name="psproj", bufs=2, space="PSUM"))
    ps_sc = ctx.enter_context(tc.tile_pool(name="pssc", bufs=2, space="PSUM"))
    ps_f = ctx.enter_context(tc.tile_pool(name="psf", bufs=4, space="PSUM"))

    # ---- constants: linear DMAs, all host-prebuilt; chunk 0 of w first
    # so the first QKV matmul's LDWEIGHTS can start as early as possible
    wq_sb = const.tile([128, NC_CHUNKS, 192], DT)
    nc.scalar.dma_start(out=wq_sb[:, 0, :], in_=wq_blk[:, 0:192])
    nc.scalar.dma_start(
        out=wq_sb[:, 1:NC_CHUNKS, :],
        in_=wq_blk[:, 192:NC_CHUNKS * 192].rearrange(
            "p (c e) -> p c e", c=NC_CHUNKS - 1))
    waug = const.tile([65, D], DT)
    nc.scalar.dma_start(out=waug[:], in_=waug_in[:, :])
    maskc = const.tile([128, 2, BW], DT)
    nc.scalar.dma_start(out=maskc[:], in_=maskc_in[:, :, :])
    ident = const.tile([64, 64], DT)
    nc.scalar.dma_start(out=ident[:], in_=ident_in[:, :])

    def emit_proj(pt):
        """x load, QKV projection and PSUM->SBUF casts.

        Emitted FIRST in each iteration so the casts sit ahead of the
        previous tail's y-copies in the scalar/vector queues -- the
        casts feed the PE's next scores/transposes, while y-copies only
        feed the (slack) output DMA.  This breaks the serial cycle
        finals -> y-copies -> casts -> scores.
        """
        # linear 16-bit load: xp[p, c, t] = x_blk[(pt*6 + c)*128 + p, t]
        xp = xplane_pool.tile([128, 2, NC_CHUNKS // 2, PT], DT, tag="xp")
        hlen = NC_CHUNKS // 2 * PT
        if pt == 0:
            # chunk-granular first load so QKV chunk 0 starts sooner
            for h in range(2):
                for c3 in range(NC_CHUNKS // 2):
                    o0 = (pt * 2 + h) * hlen + c3 * PT
                    nc.sync.dma_start(
                        out=xp[:, h, c3, :],
                        in_=x_blk[:, o0:o0 + PT],
                    )
        else:
            for h in range(2):
                o0 = (pt * 2 + h) * hlen
                nc.sync.dma_start(
                    out=xp[:, h, :, :],
                    in_=x_blk[:, o0:o0 + hlen].rearrange(
                        "p (c t) -> p c t", c=NC_CHUNKS // 2),
                )

        # qkT: [128(e = q|k), PT] = sum_c W_qk[c].T @ x[c]
        qk_ps = ps_proj.tile([128, PT], F32, tag="proj")
        v_ps = ps_proj.tile([128, PT], F32, tag="proj")
        if pt == 0:
            # warmup: interleave qk/v per chunk so each freshly-landed x
            # chunk feeds two matmuls while the next chunk's DMA streams
            for c in range(NC_CHUNKS):
                nc.tensor.matmul(
                    qk_ps[:], lhsT=wq_sb[:, c, 0:128],
                    rhs=xp[:, c // 3, c % 3, :],
                    start=(c == 0), stop=(c == NC_CHUNKS - 1),
                )
                nc.tensor.matmul(
                    v_ps[0:64, :], lhsT=wq_sb[:, c, 128:192],
                    rhs=xp[:, c // 3, c % 3, :],
                    start=(c == 0), stop=(c == NC_CHUNKS - 1),
                )
        else:
            for c in range(NC_CHUNKS):
                nc.tensor.matmul(
                    qk_ps[:], lhsT=wq_sb[:, c, 0:128],
                    rhs=xp[:, c // 3, c % 3, :],
                    start=(c == 0), stop=(c == NC_CHUNKS - 1),
                )
            for c in range(NC_CHUNKS):
                nc.tensor.matmul(
                    v_ps[0:64, :], lhsT=wq_sb[:, c, 128:192],
                    rhs=xp[:, c // 3, c % 3, :],
                    start=(c == 0), stop=(c == NC_CHUNKS - 1),
                )
        qT = qkv_pool.tile([64, PT], DT, tag="qT")
        kT = qkv_pool.tile([64, PT], DT, tag="kT")
        nc.scalar.copy(qT[:], qk_ps[0:64, :])
        nc.vector.tensor_copy(kT[:], qk_ps[64:128, :])
        vT = qkv_pool.tile([64, PT], DT, tag="vT")
        nc.vector.tensor_copy(vT[:], v_ps[0:64, :])

        return {"pt": pt, "qT": qT, "kT": kT, "vT": vT}

    def emit_attn(fr):
        """vT transposes, banded scores, exp, mask for a projected ptile."""
        qT, kT, vT = fr["qT"], fr["kT"], fr["vT"]

        # per-batch PSUM tile: scores [:, jc, 0:160] f32 + transposed-v
        # spare [:, jc, 160:192] (bitcast fp16) in the SAME bank
        vaug = vaug_pool.tile([128, nbatch_pt, 2, 65], DT, tag="vaug")
        nc.gpsimd.memset(vaug[:, :, :, 64:65], 1.0)
        sc_list = [ps_sc.tile([128, 2, BW + 32], F32, tag="sc", name=f"sc{bb}")
                   for bb in range(nbatch_pt)]
        # all transposes first, then all scores: uniform PE tile-size runs
        for bb in range(nbatch_pt):
            for jc in range(2):
                nc.tensor.transpose(
                    sc_list[bb][:, jc, BW:BW + 32].bitcast(DT),
                    vT[:, bb * N + jc * 128: bb * N + (jc + 1) * 128],
                    ident[:],
                )
        for bb in range(nbatch_pt):
            t0 = bb * N
            # banded scores, compact:
            #   jc0: keys [0,128)   x queries [0,160)
            #   jc1: keys [128,256) x queries [96,256)
            for jc, ilo in ((0, 0), (1, N - BW)):
                nc.tensor.matmul(
                    sc_list[bb][:, jc, 0:BW],
                    lhsT=kT[:, t0 + jc * 128: t0 + (jc + 1) * 128],
                    rhs=qT[:, t0 + ilo: t0 + ilo + BW], start=True, stop=True,
                )
            nc.vector.tensor_copy(vaug[:, bb, :, 0:64],
                                  sc_list[bb][:, :, BW:BW + 32].bitcast(DT))

        # one wide exp per batch, then one gpsimd band-mask multiply
        pexps = []
        for bb in range(nbatch_pt):
            pexp = exp_pool.tile([128, 2, BW], DT, tag="pexp")
            nc.scalar.activation(
                pexp[:], sc_list[bb][:, :, 0:BW],
                mybir.ActivationFunctionType.Exp, scale=float(DH) ** -0.5,
            )
            pexps.append(pexp)
        # per-jc mask halves: PV's first matmuls only need jc0 masked
        for bb in range(nbatch_pt):
            nc.gpsimd.tensor_mul(pexps[bb][:, 0, :], pexps[bb][:, 0, :],
                                 maskc[:, 0, :])
        for bb in range(nbatch_pt):
            nc.gpsimd.tensor_mul(pexps[bb][:, 1, :], pexps[bb][:, 1, :],
                                 maskc[:, 1, :])

        fr["vaug"] = vaug
        fr["pexps"] = pexps
        return fr

    def emit_tail_pv(st, final=False):
        """PV matmuls, osc eviction, sums store for a previous ptile.

        o_ps lives in the f-ring (NOT the qk/v ring): PV must never wait
        on the current ptile's qk eviction casts.
        """
        pt, vaug, pexps = st["pt"], st["vaug"], st["pexps"]
        o_ps = ps_f.tile([128, PT], F32, tag="f", name="o_ps")
        for bb in range(nbatch_pt):
            t0 = bb * N
            pexp = pexps[bb]
            # compact banded PV, 3 matmuls: jc0 [0:160), jc1 accumulates
            # the overlap [96:160) (skip_group_check: the group already
            # stopped -- hardware accumulate is just write-with-add),
            # jc1 fresh [160:256)
            nc.tensor.matmul(
                o_ps[0:65, t0 + 0:t0 + 160],
                lhsT=vaug[:, bb, 0, :], rhs=pexp[:, 0, 0:BW],
                start=True, stop=True,
            )
            nc.tensor.matmul(
                o_ps[0:65, t0 + 96:t0 + 160],
                lhsT=vaug[:, bb, 1, :], rhs=pexp[:, 1, 0:64],
                start=False, stop=True, skip_group_check=True,
            )
            nc.tensor.matmul(
                o_ps[0:65, t0 + 160:t0 + 256],
                lhsT=vaug[:, bb, 1, :], rhs=pexp[:, 1, 64:BW],
                start=True, stop=True,
            )

        # single wide osc eviction, alternating engine; split across both
        # engines at the pipeline drain (nothing to hide behind there)
        osc = osc_pool.tile([65, PT], DT, tag="osc")
        if final:
            nc.scalar.copy(osc[:, 0:PT // 2], o_ps[0:65, 0:PT // 2])
            nc.vector.tensor_copy(osc[:, PT // 2:PT], o_ps[0:65, PT // 2:PT])
        elif pt % 2 == 0:
            nc.scalar.copy(osc[:], o_ps[0:65, :])
        else:
            nc.vector.tensor_copy(osc[:], o_ps[0:65, :])
        # softmax sums (osc row 64) ship raw; the host normalizes
        nc.sync.dma_start(out=sums_out[:, pt * PT:(pt + 1) * PT],
                          in_=osc[64:65, :])
        st["osc"] = osc

    def emit_tail_f(st, half, final=False):
        """Final projection, transposed: yT[dc, t] = waug_c^T @ osc.

        waug chunks are CONSTANT lhsT weights (LDWEIGHTS never waits on
        data), all 6 matmuls are uniform full-bank 512-column ops, and
        the bias-times-sums trick still applies (host divides by sums).
        Emitted in two halves one iteration apart so the f-ring bank
        evictions always have a full qk/v section to drain behind.
        """
        pt, osc = st["pt"], st["osc"]
        if half == 0:
            y_sb = y_pool.tile([128, NC_CHUNKS, PT], DT, tag="ysb")
            st["y_sb"] = y_sb
            dcs = range(0, NC_CHUNKS // 2)
        else:
            y_sb = st["y_sb"]
            dcs = range(NC_CHUNKS // 2, NC_CHUNKS)
        for dc in dcs:
            f_ps = ps_f.tile([128, PT], F32, tag="f")
            nc.tensor.matmul(
                f_ps[:], lhsT=waug[:, dc * 128:(dc + 1) * 128],
                rhs=osc[:], start=True, stop=True,
            )
            if final:
                # drain: split each eviction across both engines and DMA
                # each chunk as soon as it lands, so the last transfer is
                # small and the receipt latency shrinks
                nc.scalar.copy(y_sb[:, dc, 0:PT // 2], f_ps[:, 0:PT // 2])
                nc.vector.tensor_copy(y_sb[:, dc, PT // 2:PT],
                                      f_ps[:, PT // 2:PT])
                eng = nc.scalar if dc == NC_CHUNKS - 1 else nc.sync
                eng.dma_start(
                    out=y[:, (pt * NC_CHUNKS + dc) * PT:
                          (pt * NC_CHUNKS + dc + 1) * PT],
                    in_=y_sb[:, dc, :],
                )
            elif dc % 2 == 0:
                nc.scalar.copy(y_sb[:, dc, :], f_ps[:])
            else:
                nc.vector.tensor_copy(y_sb[:, dc, :], f_ps[:])
        if half == 1 and not final:
            eng = nc.sync
            eng.dma_start(
                out=y[:, pt * NC_CHUNKS * PT:
                      (pt + 1) * NC_CHUNKS * PT].rearrange(
                    "p (c t) -> p c t", c=NC_CHUNKS),
                in_=y_sb[:],
            )

    # ---- software-pipelined main loop ----
    # Per iteration: proj(pt) [qk/v matmuls + casts], finals 2nd half
    # (pt-3), tail_pv(pt-2) [PV + osc + sums], attn(pt) [transposes,
    # scores, exp, mask], finals 1st half (pt-2).
    sts = [None] * npt
    for pt in range(npt):
        sts[pt] = emit_proj(pt)
        if pt >= 3:
            emit_tail_f(sts[pt - 3], half=1)
        if pt >= 2:
            emit_tail_pv(sts[pt - 2])
        emit_attn(sts[pt])
        if pt >= 2:
            emit_tail_f(sts[pt - 2], half=0)
    # drain: both PV groups first (osc evictions overlap the other
    # ptile's PE work), then both final-projection groups
    emit_tail_f(sts[npt - 3], half=1)
    emit_tail_pv(sts[npt - 2])
    emit_tail_pv(sts[npt - 1], final=True)
    emit_tail_f(sts[npt - 2], half=0)
    emit_tail_f(sts[npt - 2], half=1)
    emit_tail_f(sts[npt - 1], half=0, final=True)
    emit_tail_f(sts[npt - 1], half=1, final=True)


def build_nc(tok=TOK_FULL):
    nc = bacc.Bacc("TRN2", target_bir_lowering=False, debug=False)
    # x 16-bit, host-pre-transposed, chunk-blocked per ptile:
    # x_blk[(pt*6 + c)*128 + p, t] = cast(x)[pt*PT + t, c*128 + p]
    x_blk = nc.dram_tensor(
        "x_blk", [128, tok // PT * 2 * NC_CHUNKS // 2 * PT], DT,
        kind="ExternalInput").ap()
    wq_blk = nc.dram_tensor("wq_blk", [128, NC_CHUNKS * 192], DT,
                            kind="ExternalInput").ap()
    waug_in = nc.dram_tensor("waug", [65, D], DT, kind="ExternalInput").ap()
    maskc_in = nc.dram_tensor("maskc", [128, 2, BW], DT,
                              kind="ExternalInput").ap()
    ident_in = nc.dram_tensor("ident", [64, 64], DT, kind="ExternalInput").ap()
    # y is stored TRANSPOSED: y_blk[p, (pt, c, t)] = yraw[pt*PT+t, c*128+p]
    y = nc.dram_tensor("y", [128, tok // PT * NC_CHUNKS * PT], DT,
                       kind="ExternalOutput").ap()
    sums_out = nc.dram_tensor("sums", [1, tok], DT, kind="ExternalOutput").ap()

    from contextlib import ExitStack
    with tile.TileContext(nc) as tc:
        with ExitStack() as ctx:
            build_body(tc, x_blk, wq_blk, waug_in, maskc_in, ident_in, y,
                       sums_out, tok, ctx)
    nc.compile()
    return nc


def make_in_maps(x, w_qkv, w_out, b_out):
    w16 = np.asarray(w_qkv, dtype=NP_DT)
    wq_blk = np.ascontiguousarray(
        w16.reshape(NC_CHUNKS, 128, 192).transpose(1, 0, 2)
    ).reshape(128, NC_CHUNKS * 192)

    waug = np.zeros((65, D), dtype=NP_DT)
    waug[0:64, :] = np.asarray(w_out, dtype=NP_DT)
    waug[64, :] = np.asarray(b_out, dtype=NP_DT)

    # band masks on the compact column ranges:
    #   jc0: key j = k,     query i = c      -> keep iff |c - k| < SA
    #   jc1: key j = 128+k, query i = 96+c   -> keep iff |c - k - 32| < SA
    k = np.arange(128)[:, None]
    c = np.arange(BW)[None, :]
    maskc = np.zeros((128, 2, BW), dtype=NP_DT)
    maskc[:, 0, :] = (np.abs(c - k) < SA).astype(NP_DT)
    maskc[:, 1, :] = (np.abs(c - k - 32) < SA).astype(NP_DT)

    ident = np.eye(64, dtype=NP_DT)

    npt = TOK_FULL // PT
    in_maps = []
    for cc in range(NCORES):
        xc = np.asarray(x)[cc * BLOC:(cc + 1) * BLOC].reshape(TOK_FULL, D)
        xc16 = xc.astype(NP_DT)
        blk = np.ascontiguousarray(
            xc16.reshape(npt, PT, 2, NC_CHUNKS // 2, 128)
            .transpose(4, 0, 2, 3, 1)
        ).reshape(128, -1)
        in_maps.append({
            "x_blk": blk,
            "wq_blk": wq_blk, "waug": waug, "maskc": maskc, "ident": ident,
        })
    return in_maps


_NC_CACHE = {}


def run(x, w_qkv, w_out, b_out, trace=False, **trace_kwargs):
    if "nc" not in _NC_CACHE:
        _NC_CACHE["nc"] = build_nc()
    nc = _NC_CACHE["nc"]
    in_maps = make_in_maps(x, w_qkv, w_out, b_out)
    res = bass_utils.run_bass_kernel_spmd(
        nc, in_maps, core_ids=list(range(NCORES)), trace=trace, **trace_kwargs
    )
    npt = TOK_FULL // PT
    outs = []
    for c in range(NCORES):
        # y arrives transposed: [p, pt, dc, t] = yraw[pt*PT+t, dc*128+p]
        raw = (np.asarray(res.results[c]["y"], dtype=np.float32)
               .reshape(128, npt, NC_CHUNKS, PT).transpose(1, 3, 2, 0)
               .reshape(TOK_FULL, D))
        sums = np.asarray(res.results[c]["sums"], dtype=np.float32).reshape(
            TOK_FULL, 1)
        outs.append((raw / sums).reshape(BLOC, N, D))
    y = np.concatenate(outs, axis=0)
    return y, res


def kernel(x, w_qkv, w_out, b_out):
    y, _ = run(np.asarray(x), np.asarray(w_qkv), np.asarray(w_out),
               np.asarray(b_out))
    return y
